# revision 36
# baseline (speedup 1.0000x reference)
"""Bass/Trainium2 kernel for nn_CrossAttention_33586644254982.

Math: the cross-attention has a single KV token, so softmax over the
key axis (size 1) is exactly 1.0 and the attention output equals V
broadcast over all N query positions. The full module therefore reduces to

    out[b, n, :] = (freq_token[b] @ Wv.T + bv) @ Wo.T + bo     (independent of n)

Q/K projections and spatial_tokens do not affect the output at all.
The two consecutive linear layers are folded into one (offline weight
preprocessing, done host-side in float64):

    Wc = Wo @ Wv          [C, CFD]
    bc = Wo @ bv + bo     [C]
    out[b, n, :] = freq_token[b] @ Wc.T + bc

Strategy: data-parallel over B (16 batches -> 2 per core on 8 cores).
Per core, the kernel computes the matmul directly in BROADCAST form:
the stationary operand is ft_b's k-chunk replicated across all 128
M-columns (prepared host-side), so each PSUM result tile [128, 384]
holds the O row already broadcast across partitions - no separate
row-extract / partition-broadcast / copy chain. The DVE tensor_add
that moves PSUM->SBUF adds the host-broadcast folded bias (f16).
f16 operands: one PE pass per matmul (fp32 needs two) - the PE runs
at its cold ~1.1 GHz clock this early, so pass count dominates the
compute phase. h=0's chain is ordered to finish two passes early so
its bias-add overlaps the h=1 tail.

Loads: one whole-tile DMA each (a single HWDGE completion semaphore
posts ~1.5 us after the data lands; splitting a tile doubles that
wait), weights packed host-side so each partition's rows for two
k-chunks are one contiguous 3 KiB descriptor (the rings process ~1
descriptor per 14 ns, the load bottleneck).

Output is stored in f16 - it halves the dominant HBM store traffic
(12 MiB/core instead of 24); the host upcasts to f32 during unshard.
Total error vs the fp32 reference is ~3.6e-4 l2 (fold in f64, f16
matmul operands, f16 output rounding) against a 2e-2 budget. The DVE
replicates each batch's output row 4x in the free dim and the shard
streams out as 6 KiB descriptors (4 output rows per partition per
DMA; 3 KiB descriptors cap at ~396 GB/s on descriptor rate),
alternating between the SP and ACT HWDGE rings (~206 GB/s each) with
one block per batch on the SWDGE queue to close the gap to the
435 GB/s HBM write cap.

Measured: ~50-54 us when this core's HBM share is uncontended
(~413 GB/s store rate), ~56-59 us when all 8 cores' store phases
fully contend (~345 GB/s). Breakdown: ~7 us fixed framework preamble,
~9-10 us loads + 16 matmul passes + bias-adds (semaphore-latency
dominated), ~29-37 us store stream, ~2.7 us drain/teardown.
"""

import numpy as np

# Problem shapes (hardcoded per contract - kernel.py is self-contained).
B, N, C, CFD = 16, 4096, 768, 512
N_CORES = 8
BPC = B // N_CORES  # batches per core = 2
P = 128
KA = CFD // P       # k-chunks for the matmul = 4
PAIRS = KA // 2     # k-chunk pairs packed per load descriptor = 2
K_REP = 4           # output rows per partition per store descriptor
T = N // (K_REP * P)  # output DMAs per batch = 16
H = 64              # partition-split point for ring load balancing

_CACHE = {}


def _build():
    from concourse import bacc, mybir
    from concourse.tile import TileContext

    f32 = mybir.dt.float32
    f16 = mybir.dt.float16
    nc = bacc.Bacc("TRN2", debug=False, num_devices=N_CORES)

    ftb = nc.dram_tensor("ftb", [P, KA, BPC, P], f16, kind="ExternalInput").ap()
    wcp = nc.dram_tensor("wcp", [P, PAIRS, 2, C], f16, kind="ExternalInput").ap()
    bcb = nc.dram_tensor("bcb", [P, C], f16, kind="ExternalInput").ap()
    # Output in f16: halves the dominant HBM store traffic (12 MiB/core
    # instead of 24); the host upcasts to f32 during unshard. Rounding
    # adds ~2.4e-4 relative error against a 2e-2 budget.
    out = nc.dram_tensor("out", [BPC, N, C], f16, kind="ExternalOutput").ap()

    with TileContext(nc) as tc:
        with (
            tc.tile_pool(name="consts", bufs=1) as consts,
            tc.tile_pool(name="weights", bufs=1) as weights,
            tc.tile_pool(name="repl", bufs=2) as replp,
            tc.tile_pool(name="ps_b", bufs=1, space="PSUM") as ps_b,
            tc.tile_pool(name="ps_warm", bufs=1, space="PSUM") as ps_warm,
        ):
            # Loads: one whole-tile DMA each (a single completion
            # semaphore per tile posts ~1.5 us after the data lands;
            # partition-split halves doubled that wait), interleaved
            # across the two HWDGE rings by need-time: ft + chunk-pair 0
            # gate the first matmul pass, chunk-pair 1 the fifth, the
            # broadcast bias only the PSUM->SBUF adds.
            ft_sb = consts.tile([P, KA, BPC, P], f16)
            wc_sb = weights.tile([P, PAIRS, 2, C], f16)
            bc_sb = consts.tile([P, C], f16)
            nc.sync.dma_start(out=ft_sb, in_=ftb)
            nc.scalar.dma_start(out=wc_sb[:, 0], in_=wcp[:, 0])
            nc.sync.dma_start(out=wc_sb[:, 1], in_=wcp[:, 1])
            nc.scalar.dma_start(out=bc_sb, in_=bcb)

            # Short PE warm-up on zeroed f16 scratch while loads land.
            dum_l = consts.tile([P, P], f16)
            nc.vector.memset(dum_l, 0.0)
            dum_r = consts.tile([P, 512], f16)
            nc.vector.memset(dum_r, 0.0)
            ps_w = ps_warm.tile([P, 512], f32)
            for _ in range(4):
                nc.tensor.matmul(ps_w, dum_l, dum_r, start=True, stop=True)

            # r4[p, j] = sum_k ftb[k, *, b, p] Wc[j, k] + bc[j]  (same for
            # every p). Two PSUM-bank halves (N=384) x 4 k-chunks per
            # batch; batch 0 fully first so its stores start earliest.
            NS1 = C // 2  # 384
            # All stores use K_REP-row 6 KiB descriptors: 3 KiB ones cap
            # at ~396 GB/s aggregate on descriptor rate. (A free-dim
            # stride-0 broadcast source would skip the replication copies
            # but lowers to 4x 1.5 KiB descriptors per partition -
            # descriptor-capped stores, measured net loss.)
            outq = out.rearrange("b (t p q) c -> b t p (q c)", p=P, q=K_REP)
            engines = [nc.sync, nc.scalar]
            di = 0
            for b in range(BPC):
                pss = [
                    ps_b.tile([P, NS1], f32, name=f"ps_b{b}h{h}")
                    for h in range(2)
                ]
                # h=0's chain finishes two passes early (order below) so
                # its bias-add overlaps the h=1 chain's tail.
                for a, h in (
                    (0, 0), (0, 1), (1, 0), (1, 1),
                    (2, 0), (3, 0), (2, 1), (3, 1),
                ):
                    nc.tensor.matmul(
                        pss[h],
                        ft_sb[:, a, b, :],
                        wc_sb[:, a // 2, a % 2, h * NS1 : (h + 1) * NS1],
                        start=(a == 0),
                        stop=(a == KA - 1),
                    )
                r4 = replp.tile([P, K_REP, C], f16)
                for h in range(2):
                    sl = slice(h * NS1, (h + 1) * NS1)
                    nc.vector.tensor_add(r4[:, 0, sl], pss[h], bc_sb[:, sl])
                # Replication copies on DVE (349 ns each at the 2x f16
                # rate; ACT's activation-copy takes 934 ns). b=1's adds
                # can't be scheduled ahead of these - their PSUM deps
                # aren't ready yet.
                for rep in range(1, K_REP):
                    nc.vector.tensor_copy(r4[:, rep, :], r4[:, 0, :])
                r4_flat = r4.rearrange("p r c -> p (r c)")
                for t in range(N // (K_REP * P)):
                    engines[di % 2].dma_start(out=outq[b, t], in_=r4_flat)
                    di += 1

    nc.compile()
    return nc


def _get_nc():
    if "nc" not in _CACHE:
        _CACHE["nc"] = _build()
    return _CACHE["nc"]


def _install_ntff_hook():
    """Provide antenv.axon_hooks if the image lacks it (profiling only)."""
    import sys
    import types

    try:
        from antenv.axon_hooks import get_axon_ntff_profile_hook  # noqa: F401

        return
    except ImportError:
        pass
    try:
        import antenv
        from trn_agent_boot.trn_boot import _ntff_profile_via_ctypes

        hook = _ntff_profile_via_ctypes("/opt/axon/libaxon_pjrt.so")
        mod = types.ModuleType("antenv.axon_hooks")
        mod.get_axon_ntff_profile_hook = lambda: hook
        mod.set_axon_ntff_profile_hook = lambda h: None
        sys.modules["antenv.axon_hooks"] = mod
        antenv.axon_hooks = mod
    except Exception as e:  # pragma: no cover - profiling is best-effort
        print(f"ntff hook install failed ({e}); tracing disabled", file=sys.stderr)


def _run(inputs, trace=False):
    from concourse import bass_utils

    if trace:
        _install_ntff_hook()
        # Zero-egress container: skip the artifact upload, keep files local.
        bass_utils.upload_artifacts = lambda tmpdir: tmpdir

    nc = _get_nc()
    ft = np.asarray(inputs["freq_token"], np.float32)
    # Fold the two linear layers (weight preprocessing, float64 for
    # accuracy): out_row = ft @ (Wo @ Wv).T + (Wo @ bv + bo).
    Wv64 = np.asarray(inputs["Wv"], np.float64)
    Wo64 = np.asarray(inputs["Wo"], np.float64)
    bv64 = np.asarray(inputs["bv"], np.float64)
    bo64 = np.asarray(inputs["bo"], np.float64)
    WcT = (Wv64.T @ Wo64.T).astype(np.float16)  # [CFD, C]
    bc = (Wo64 @ bv64 + bo64).astype(np.float16)  # [C]

    # wcp[p, q, j, c] = WcT[(2q+j)*128 + p, c]: each partition's rows for
    # a chunk-pair are contiguous -> 3 KiB load descriptors.
    wcp = np.ascontiguousarray(
        WcT.reshape(PAIRS, 2, P, C).transpose(2, 0, 1, 3)
    )
    bcb = np.ascontiguousarray(np.broadcast_to(bc, (P, C)))

    in_maps = []
    for i in range(N_CORES):
        ft_loc = ft[BPC * i : BPC * (i + 1)]  # [BPC, CFD]
        # ftb[k, a, b, m] = ft_loc[b, a*128 + k] for every m (stationary
        # operand replicated across the 128 M-columns = broadcast form).
        ftb = np.ascontiguousarray(
            np.broadcast_to(
                ft_loc.T.reshape(KA, P, BPC).transpose(1, 0, 2)[:, :, :, None],
                (P, KA, BPC, P),
            ).astype(np.float16)
        )
        in_maps.append({"ftb": ftb, "wcp": wcp, "bcb": bcb})
    res = bass_utils.run_bass_kernel_spmd(
        nc, in_maps, core_ids=list(range(N_CORES)), trace=trace
    )
    out = np.concatenate(
        [m["out"].astype(np.float32) for m in res.results], axis=0
    )
    return out, res


def kernel(**inputs):
    out, _ = _run(inputs, trace=False)
    return out


# revision 51
# speedup vs baseline: 1.0313x; 1.0313x over previous
"""Bass/Trainium2 kernel for nn_CrossAttention_33586644254982.

Math: the cross-attention has a single KV token, so softmax over the
key axis (size 1) is exactly 1.0 and the attention output equals V
broadcast over all N query positions. The full module therefore reduces to

    out[b, n, :] = (freq_token[b] @ Wv.T + bv) @ Wo.T + bo     (independent of n)

Q/K projections and spatial_tokens do not affect the output at all.
The two consecutive linear layers are folded into one (offline weight
preprocessing, done host-side in float64):

    Wc = Wo @ Wv          [C, CFD]
    bc = Wo @ bv + bo     [C]
    out[b, n, :] = freq_token[b] @ Wc.T + bc

Strategy: data-parallel over B (16 batches -> 2 per core on 8 cores).
Per core, the kernel computes the matmul directly in BROADCAST form:
the stationary operand is ft_b's k-chunk as a free-dim stride-0 AP
broadcast across all 128 M-columns (LDWEIGHTS accepts it; partition
stride-0 is rejected), so each PSUM result tile [128, 384] holds the
O row already broadcast across partitions - no separate row-extract /
partition-broadcast / copy chain. The DVE tensor_add that moves
PSUM->SBUF adds the host-broadcast folded bias (f16).
f16 operands: one PE pass per matmul (fp32 needs two) - the PE runs
at its cold ~1.1 GHz clock this early, so pass count dominates the
compute phase. h=0's chain is ordered to finish two passes early so
its bias-add overlaps the h=1 tail.

Loads: one whole-tile DMA each (a single HWDGE completion semaphore
posts ~1.5 us after the data lands; splitting a tile doubles that
wait), weights packed host-side so each partition's rows for two
k-chunks are one contiguous 3 KiB descriptor (the rings process ~1
descriptor per 14 ns, the load bottleneck).

Output is stored in f16 - it halves the dominant HBM store traffic
(12 MiB/core instead of 24); the host upcasts to f32 during unshard.
Total error vs the fp32 reference is ~3.6e-4 l2 (fold in f64, f16
matmul operands, f16 output rounding) against a 2e-2 budget. The DVE
replicates each batch's output row 4x in the free dim and the shard
streams out as 6 KiB descriptors (4 output rows per partition per
DMA; 3 KiB descriptors cap at ~396 GB/s aggregate on descriptor
rate), alternating between the SP and ACT HWDGE rings (~206 GB/s
each). Adding the SWDGE queue as a third store path measured STRICTLY
WORSE in window-paired A/B (degrades HBM write arbitration); 12 KiB
descriptors reach 422 GB/s but the extra replication copies offset
the gain.

Measured: ~50-54 us when this core's HBM share is uncontended
(~413 GB/s store rate), ~56-59 us when all 8 cores' store phases
fully contend (~345 GB/s). Breakdown: ~7 us fixed framework preamble,
~9-10 us loads + 16 matmul passes + bias-adds (semaphore-latency
dominated), ~29-37 us store stream, ~2.7 us drain/teardown.
"""

import numpy as np

# Problem shapes (hardcoded per contract - kernel.py is self-contained).
B, N, C, CFD = 16, 4096, 768, 512
N_CORES = 8
BPC = B // N_CORES  # batches per core = 2
P = 128
KA = CFD // P       # k-chunks for the matmul = 4
PAIRS = KA // 2     # k-chunk pairs packed per load descriptor = 2
K_REP = 4           # output rows per partition per store descriptor
T = N // (K_REP * P)  # output DMAs per batch = 8

_CACHE = {}


def _build():
    from concourse import bacc, mybir
    from concourse.tile import TileContext

    f32 = mybir.dt.float32
    f16 = mybir.dt.float16
    nc = bacc.Bacc("TRN2", debug=False, num_devices=N_CORES)

    # wcf = the tiny ft column (KA*BPC f16 values per partition) with Wc
    # chunks a0|a1 appended: [ftc | a0 | a1] per partition row. Loaded as
    # TWO same-ring DMAs split after a0, so the first two matmul passes
    # get their own (earlier) completion semaphore - they only need
    # ftc + a0, not the whole pair. The stationary operand is a free-dim
    # stride-0 broadcast view of the ft column - no host-side 128x
    # replication, no separate 256 KiB ftb load.
    NFT = KA * BPC  # 8 ft values per partition, at the row start
    wcf = nc.dram_tensor(
        "wcf", [P, NFT + PAIRS * C], f16, kind="ExternalInput"
    ).ap()
    wcp = nc.dram_tensor("wcp", [P, PAIRS, C], f16, kind="ExternalInput").ap()
    bcb = nc.dram_tensor("bcb", [P, C], f16, kind="ExternalInput").ap()
    # Output in f16: halves the dominant HBM store traffic (12 MiB/core
    # instead of 24); the host upcasts to f32 during unshard. Rounding
    # adds ~2.4e-4 relative error against a 2e-2 budget.
    out = nc.dram_tensor("out", [BPC, N, C], f16, kind="ExternalOutput").ap()

    with TileContext(nc) as tc:
        with (
            tc.tile_pool(name="consts", bufs=1) as consts,
            tc.tile_pool(name="weights", bufs=1) as weights,
            tc.tile_pool(name="repl", bufs=2) as replp,
            tc.tile_pool(name="ps_b", bufs=1, space="PSUM") as ps_b,
            tc.tile_pool(name="ps_warm", bufs=1, space="PSUM") as ps_warm,
        ):
            # Loads: one whole-tile DMA each (a single completion
            # semaphore per tile posts ~1.5 us after the data lands;
            # partition-split halves doubled that wait), interleaved
            # across the two HWDGE rings by need-time: wcf (chunk-pair 0
            # + ft column) gates the first matmul pass, chunk-pair 1 the
            # fifth, the broadcast bias only the PSUM->SBUF adds. With
            # ft folded into wcf, pair 1 rides the sync ring alone and
            # its semaphore posts ~1.4 us earlier.
            wcf_sb = weights.tile([P, NFT + PAIRS * C], f16)
            wc1_sb = weights.tile([P, PAIRS, C], f16)
            bc_sb = consts.tile([P, C], f16)
            SP1 = NFT + C  # split point: [ftc | a0] then [a1]
            nc.sync.dma_start(out=wc1_sb, in_=wcp)
            nc.scalar.dma_start(out=wcf_sb[:, :SP1], in_=wcf[:, :SP1])
            nc.scalar.dma_start(out=wcf_sb[:, SP1:], in_=wcf[:, SP1:])
            nc.scalar.dma_start(out=bc_sb, in_=bcb)

            # Short PE warm-up on zeroed f16 scratch while loads land.
            dum_l = consts.tile([P, P], f16)
            nc.vector.memset(dum_l, 0.0)
            dum_r = consts.tile([P, 512], f16)
            nc.vector.memset(dum_r, 0.0)
            ps_w = ps_warm.tile([P, 512], f32)
            for _ in range(4):
                nc.tensor.matmul(ps_w, dum_l, dum_r, start=True, stop=True)

            # r4[p, j] = sum_k ftb[k, *, b, p] Wc[j, k] + bc[j]  (same for
            # every p). Two PSUM-bank halves (N=384) x 4 k-chunks per
            # batch; batch 0 fully first so its stores start earliest.
            NS1 = C // 2  # 384
            # All stores use K_REP-row 6 KiB descriptors: 3 KiB ones cap
            # at ~396 GB/s aggregate on descriptor rate. (A free-dim
            # stride-0 broadcast source would skip the replication copies
            # but lowers to 4x 1.5 KiB descriptors per partition -
            # descriptor-capped stores, measured net loss.)
            outq = out.rearrange("b (t p q) c -> b t p (q c)", p=P, q=K_REP)
            engines = [nc.sync, nc.scalar]
            di = 0
            for b in range(BPC):
                pss = [
                    ps_b.tile([P, NS1], f32, name=f"ps_b{b}h{h}")
                    for h in range(2)
                ]
                # h=0's chain finishes two passes early (order below) so
                # its bias-add overlaps the h=1 chain's tail.
                for a, h in (
                    (0, 0), (0, 1), (1, 0), (1, 1),
                    (2, 0), (3, 0), (2, 1), (3, 1),
                ):
                    fto = a * BPC + b
                    if a < 2:
                        lo = NFT + a * C + h * NS1
                        rhs = wcf_sb[:, lo : lo + NS1]
                    else:
                        rhs = wc1_sb[:, a - 2, h * NS1 : (h + 1) * NS1]
                    nc.tensor.matmul(
                        pss[h],
                        wcf_sb[:, fto : fto + 1].broadcast_to([P, P]),
                        rhs,
                        start=(a == 0),
                        stop=(a == KA - 1),
                    )
                r4 = replp.tile([P, K_REP, C], f16)
                for h in range(2):
                    sl = slice(h * NS1, (h + 1) * NS1)
                    nc.vector.tensor_add(r4[:, 0, sl], pss[h], bc_sb[:, sl])
                # Replication copies on DVE (349 ns each at the 2x f16
                # rate; ACT's activation-copy takes 934 ns). b=1's adds
                # can't be scheduled ahead of these - their PSUM deps
                # aren't ready yet.
                for rep in range(1, K_REP):
                    nc.vector.tensor_copy(r4[:, rep, :], r4[:, 0, :])
                r4_flat = r4.rearrange("p r c -> p (r c)")
                for t in range(N // (K_REP * P)):
                    engines[di % 2].dma_start(out=outq[b, t], in_=r4_flat)
                    di += 1

    nc.compile()
    return nc


def _get_nc():
    if "nc" not in _CACHE:
        _CACHE["nc"] = _build()
    return _CACHE["nc"]


def _install_ntff_hook():
    """Provide antenv.axon_hooks if the image lacks it (profiling only)."""
    import sys
    import types

    try:
        from antenv.axon_hooks import get_axon_ntff_profile_hook  # noqa: F401

        return
    except ImportError:
        pass
    try:
        import antenv
        from trn_agent_boot.trn_boot import _ntff_profile_via_ctypes

        hook = _ntff_profile_via_ctypes("/opt/axon/libaxon_pjrt.so")
        mod = types.ModuleType("antenv.axon_hooks")
        mod.get_axon_ntff_profile_hook = lambda: hook
        mod.set_axon_ntff_profile_hook = lambda h: None
        sys.modules["antenv.axon_hooks"] = mod
        antenv.axon_hooks = mod
    except Exception as e:  # pragma: no cover - profiling is best-effort
        print(f"ntff hook install failed ({e}); tracing disabled", file=sys.stderr)


def _run(inputs, trace=False):
    from concourse import bass_utils

    if trace:
        _install_ntff_hook()
        # Zero-egress container: skip the artifact upload, keep files local.
        bass_utils.upload_artifacts = lambda tmpdir: tmpdir

    nc = _get_nc()
    ft = np.asarray(inputs["freq_token"], np.float32)
    # Fold the two linear layers (weight preprocessing, float64 for
    # accuracy): out_row = ft @ (Wo @ Wv).T + (Wo @ bv + bo).
    Wv64 = np.asarray(inputs["Wv"], np.float64)
    Wo64 = np.asarray(inputs["Wo"], np.float64)
    bv64 = np.asarray(inputs["bv"], np.float64)
    bo64 = np.asarray(inputs["bo"], np.float64)
    WcT = (Wv64.T @ Wo64.T).astype(np.float16)  # [CFD, C]
    bc = (Wo64 @ bv64 + bo64).astype(np.float16)  # [C]

    # wc_all[p, a, c] = WcT[a*128 + p, c]: each partition's rows for a
    # chunk-pair are contiguous -> 3 KiB load descriptors. Chunks a0/a1
    # go into wcf with the per-core ft column appended; a2/a3 into wcp.
    wc_all = WcT.reshape(KA, P, C).transpose(1, 0, 2)  # [P, KA, C]
    wcp = np.ascontiguousarray(wc_all[:, PAIRS:])  # [P, 2, C]
    wc01 = wc_all[:, :PAIRS].reshape(P, PAIRS * C)
    bcb = np.ascontiguousarray(np.broadcast_to(bc, (P, C)))

    in_maps = []
    for i in range(N_CORES):
        ft_loc = ft[BPC * i : BPC * (i + 1)]  # [BPC, CFD]
        # ftc[k, a, b] = ft_loc[b, a*128 + k]; the kernel broadcasts it
        # across the 128 stationary M-columns via a stride-0 view.
        ftc = (
            ft_loc.T.reshape(KA, P, BPC)
            .transpose(1, 0, 2)
            .reshape(P, KA * BPC)
            .astype(np.float16)
        )
        wcf = np.ascontiguousarray(np.concatenate([ftc, wc01], axis=1))
        in_maps.append({"wcf": wcf, "wcp": wcp, "bcb": bcb})
    res = bass_utils.run_bass_kernel_spmd(
        nc, in_maps, core_ids=list(range(N_CORES)), trace=trace
    )
    out = np.concatenate(
        [m["out"].astype(np.float32) for m in res.results], axis=0
    )
    return out, res


def kernel(**inputs):
    out, _ = _run(inputs, trace=False)
    return out


# revision 54
# speedup vs baseline: 1.1041x; 1.0706x over previous
"""Bass/Trainium2 kernel for nn_CrossAttention_33586644254982.

Math: the cross-attention has a single KV token, so softmax over the
key axis (size 1) is exactly 1.0 and the attention output equals V
broadcast over all N query positions. The full module therefore reduces to

    out[b, n, :] = (freq_token[b] @ Wv.T + bv) @ Wo.T + bo     (independent of n)

Q/K projections and spatial_tokens do not affect the output at all.
The two consecutive linear layers are folded into one (offline weight
preprocessing, done host-side in float64):

    Wc = Wo @ Wv          [C, CFD]
    bc = Wo @ bv + bo     [C]
    out[b, n, :] = freq_token[b] @ Wc.T + bc

Strategy: data-parallel over B (16 batches -> 2 per core on 8 cores).
Per core, the kernel computes the matmul directly in BROADCAST form:
the stationary operand is ft_b's k-chunk as a free-dim stride-0 AP
broadcast across all 128 M-columns (LDWEIGHTS accepts it; partition
stride-0 is rejected), so each PSUM result tile [128, 384] holds the
O row already broadcast across partitions - no separate row-extract /
partition-broadcast / copy chain. The DVE tensor_add that moves
PSUM->SBUF adds the host-broadcast folded bias (f16).
f16 operands: one PE pass per matmul (fp32 needs two) - the PE runs
at its cold ~1.1 GHz clock this early, so pass count dominates the
compute phase. h=0's chain is ordered to finish two passes early so
its bias-add overlaps the h=1 tail.

Loads: one whole-tile DMA each (a single HWDGE completion semaphore
posts ~1.5 us after the data lands; splitting a tile doubles that
wait), weights packed host-side so each partition's rows for two
k-chunks are one contiguous 3 KiB descriptor (the rings process ~1
descriptor per 14 ns, the load bottleneck).

Output is stored in f16 - it halves the dominant HBM store traffic
(12 MiB/core instead of 24); the host upcasts to f32 during unshard.
Total error vs the fp32 reference is ~3.6e-4 l2 (fold in f64, f16
matmul operands, f16 output rounding) against a 2e-2 budget. The DVE
replicates each batch's output row 4x in the free dim and the shard
streams out as 6 KiB descriptors (4 output rows per partition per
DMA; 3 KiB descriptors cap at ~396 GB/s aggregate on descriptor
rate), alternating between the SP and ACT HWDGE rings (~206 GB/s
each). Adding the SWDGE queue as a third store path measured STRICTLY
WORSE in window-paired A/B (degrades HBM write arbitration); 12 KiB
descriptors reach 422 GB/s but the extra replication copies offset
the gain.

Measured: ~50-54 us when this core's HBM share is uncontended
(~413 GB/s store rate), ~56-59 us when all 8 cores' store phases
fully contend (~345 GB/s). Breakdown: ~7 us fixed framework preamble,
~9-10 us loads + 16 matmul passes + bias-adds (semaphore-latency
dominated), ~29-37 us store stream, ~2.7 us drain/teardown.
"""

import numpy as np

# Problem shapes (hardcoded per contract - kernel.py is self-contained).
B, N, C, CFD = 16, 4096, 768, 512
N_CORES = 8
BPC = B // N_CORES  # batches per core = 2
P = 128
KA = CFD // P       # k-chunks for the matmul = 4
PAIRS = KA // 2     # k-chunk pairs packed per load descriptor = 2
K_REP = 4           # output rows per partition per store descriptor
T = N // (K_REP * P)  # output DMAs per batch = 8

_CACHE = {}


def _build():
    from concourse import bacc, mybir
    from concourse.tile import TileContext

    f32 = mybir.dt.float32
    f16 = mybir.dt.float16
    nc = bacc.Bacc("TRN2", debug=False, num_devices=N_CORES)

    # wcf = the tiny ft column (KA*BPC f16 values per partition) with Wc
    # chunks a0|a1 appended: [ftc | a0 | a1] per partition row. Loaded as
    # TWO same-ring DMAs split after a0, so the first two matmul passes
    # get their own (earlier) completion semaphore - they only need
    # ftc + a0, not the whole pair. The stationary operand is a free-dim
    # stride-0 broadcast view of the ft column - no host-side 128x
    # replication, no separate 256 KiB ftb load.
    NFT = KA * BPC  # 8 ft values per partition, at the row start
    wcf = nc.dram_tensor(
        "wcf", [P, NFT + PAIRS * C], f16, kind="ExternalInput"
    ).ap()
    wcp = nc.dram_tensor("wcp", [P, PAIRS, C], f16, kind="ExternalInput").ap()
    bcb = nc.dram_tensor("bcb", [P, C], f16, kind="ExternalInput").ap()
    # Output in f16: halves the dominant HBM store traffic (12 MiB/core
    # instead of 24); the host upcasts to f32 during unshard. Rounding
    # adds ~2.4e-4 relative error against a 2e-2 budget.
    out = nc.dram_tensor("out", [BPC, N, C], f16, kind="ExternalOutput").ap()

    with TileContext(nc) as tc:
        with (
            tc.tile_pool(name="consts", bufs=1) as consts,
            tc.tile_pool(name="weights", bufs=1) as weights,
            tc.tile_pool(name="repl", bufs=2) as replp,
            tc.tile_pool(name="ps_b", bufs=1, space="PSUM") as ps_b,
            tc.tile_pool(name="ps_warm", bufs=1, space="PSUM") as ps_warm,
        ):
            # Loads: one whole-tile DMA each (a single completion
            # semaphore per tile posts ~1.5 us after the data lands;
            # partition-split halves doubled that wait), interleaved
            # across the two HWDGE rings by need-time: wcf (chunk-pair 0
            # + ft column) gates the first matmul pass, chunk-pair 1 the
            # fifth, the broadcast bias only the PSUM->SBUF adds. With
            # ft folded into wcf, pair 1 rides the sync ring alone and
            # its semaphore posts ~1.4 us earlier.
            wcf_sb = weights.tile([P, NFT + PAIRS * C], f16)
            wc1_sb = weights.tile([P, PAIRS, C], f16)
            bc_sb = consts.tile([P, C], f16)
            SP1 = NFT + C  # split point: [ftc | a0] then [a1]
            nc.sync.dma_start(out=wc1_sb, in_=wcp)
            nc.scalar.dma_start(out=wcf_sb[:, :SP1], in_=wcf[:, :SP1])
            nc.scalar.dma_start(out=wcf_sb[:, SP1:], in_=wcf[:, SP1:])
            nc.scalar.dma_start(out=bc_sb, in_=bcb)

            # Short PE warm-up on zeroed f16 scratch while loads land.
            dum_l = consts.tile([P, P], f16)
            nc.vector.memset(dum_l, 0.0)
            dum_r = consts.tile([P, 512], f16)
            nc.vector.memset(dum_r, 0.0)
            ps_w = ps_warm.tile([P, 512], f32)
            for _ in range(4):
                nc.tensor.matmul(ps_w, dum_l, dum_r, start=True, stop=True)

            # r4[p, j] = sum_k ftb[k, *, b, p] Wc[j, k] + bc[j]  (same for
            # every p). Two PSUM-bank halves (N=384) x 4 k-chunks per
            # batch; batch 0 fully first so its stores start earliest.
            NS1 = C // 2  # 384
            # All stores use K_REP-row 6 KiB descriptors: 3 KiB ones cap
            # at ~396 GB/s aggregate on descriptor rate. (A free-dim
            # stride-0 broadcast source would skip the replication copies
            # but lowers to 4x 1.5 KiB descriptors per partition -
            # descriptor-capped stores, measured net loss.)
            outq = out.rearrange("b (t p q) c -> b t p (q c)", p=P, q=K_REP)
            engines = [nc.sync, nc.scalar]
            di = 0
            for b in range(BPC):
                pss = [
                    ps_b.tile([P, NS1], f32, name=f"ps_b{b}h{h}")
                    for h in range(2)
                ]
                # Pass order matched to semaphore arrival: a2/a3 (wcp,
                # posts ~11.5us) at passes 3-4, a1 (second wcf piece,
                # posts ~12.1us and used to stall pass 3 by ~0.9us) at
                # pass 5. h=0's stop lands at pass 6 so its bias-add
                # overlaps the tail. Accumulation order is free.
                for a, h in (
                    (0, 0), (0, 1), (2, 0), (2, 1),
                    (1, 0), (3, 0), (1, 1), (3, 1),
                ):
                    fto = a * BPC + b
                    if a < 2:
                        lo = NFT + a * C + h * NS1
                        rhs = wcf_sb[:, lo : lo + NS1]
                    else:
                        rhs = wc1_sb[:, a - 2, h * NS1 : (h + 1) * NS1]
                    nc.tensor.matmul(
                        pss[h],
                        wcf_sb[:, fto : fto + 1].broadcast_to([P, P]),
                        rhs,
                        start=(a == 0),
                        stop=(a == KA - 1),
                    )
                r4 = replp.tile([P, K_REP, C], f16)
                for h in range(2):
                    sl = slice(h * NS1, (h + 1) * NS1)
                    nc.vector.tensor_add(r4[:, 0, sl], pss[h], bc_sb[:, sl])
                # Replication copies on DVE (349 ns each at the 2x f16
                # rate; ACT's activation-copy takes 934 ns). b=1's adds
                # can't be scheduled ahead of these - their PSUM deps
                # aren't ready yet.
                for rep in range(1, K_REP):
                    nc.vector.tensor_copy(r4[:, rep, :], r4[:, 0, :])
                r4_flat = r4.rearrange("p r c -> p (r c)")
                for t in range(N // (K_REP * P)):
                    engines[di % 2].dma_start(out=outq[b, t], in_=r4_flat)
                    di += 1

    nc.compile()
    return nc


def _get_nc():
    if "nc" not in _CACHE:
        _CACHE["nc"] = _build()
    return _CACHE["nc"]


def _install_ntff_hook():
    """Provide antenv.axon_hooks if the image lacks it (profiling only)."""
    import sys
    import types

    try:
        from antenv.axon_hooks import get_axon_ntff_profile_hook  # noqa: F401

        return
    except ImportError:
        pass
    try:
        import antenv
        from trn_agent_boot.trn_boot import _ntff_profile_via_ctypes

        hook = _ntff_profile_via_ctypes("/opt/axon/libaxon_pjrt.so")
        mod = types.ModuleType("antenv.axon_hooks")
        mod.get_axon_ntff_profile_hook = lambda: hook
        mod.set_axon_ntff_profile_hook = lambda h: None
        sys.modules["antenv.axon_hooks"] = mod
        antenv.axon_hooks = mod
    except Exception as e:  # pragma: no cover - profiling is best-effort
        print(f"ntff hook install failed ({e}); tracing disabled", file=sys.stderr)


def _run(inputs, trace=False):
    from concourse import bass_utils

    if trace:
        _install_ntff_hook()
        # Zero-egress container: skip the artifact upload, keep files local.
        bass_utils.upload_artifacts = lambda tmpdir: tmpdir

    nc = _get_nc()
    ft = np.asarray(inputs["freq_token"], np.float32)
    # Fold the two linear layers (weight preprocessing, float64 for
    # accuracy): out_row = ft @ (Wo @ Wv).T + (Wo @ bv + bo).
    Wv64 = np.asarray(inputs["Wv"], np.float64)
    Wo64 = np.asarray(inputs["Wo"], np.float64)
    bv64 = np.asarray(inputs["bv"], np.float64)
    bo64 = np.asarray(inputs["bo"], np.float64)
    WcT = (Wv64.T @ Wo64.T).astype(np.float16)  # [CFD, C]
    bc = (Wo64 @ bv64 + bo64).astype(np.float16)  # [C]

    # wc_all[p, a, c] = WcT[a*128 + p, c]: each partition's rows for a
    # chunk-pair are contiguous -> 3 KiB load descriptors. Chunks a0/a1
    # go into wcf with the per-core ft column appended; a2/a3 into wcp.
    wc_all = WcT.reshape(KA, P, C).transpose(1, 0, 2)  # [P, KA, C]
    wcp = np.ascontiguousarray(wc_all[:, PAIRS:])  # [P, 2, C]
    wc01 = wc_all[:, :PAIRS].reshape(P, PAIRS * C)
    bcb = np.ascontiguousarray(np.broadcast_to(bc, (P, C)))

    in_maps = []
    for i in range(N_CORES):
        ft_loc = ft[BPC * i : BPC * (i + 1)]  # [BPC, CFD]
        # ftc[k, a, b] = ft_loc[b, a*128 + k]; the kernel broadcasts it
        # across the 128 stationary M-columns via a stride-0 view.
        ftc = (
            ft_loc.T.reshape(KA, P, BPC)
            .transpose(1, 0, 2)
            .reshape(P, KA * BPC)
            .astype(np.float16)
        )
        wcf = np.ascontiguousarray(np.concatenate([ftc, wc01], axis=1))
        in_maps.append({"wcf": wcf, "wcp": wcp, "bcb": bcb})
    res = bass_utils.run_bass_kernel_spmd(
        nc, in_maps, core_ids=list(range(N_CORES)), trace=trace
    )
    out = np.concatenate(
        [m["out"].astype(np.float32) for m in res.results], axis=0
    )
    return out, res


def kernel(**inputs):
    out, _ = _run(inputs, trace=False)
    return out
